# revision 1
# baseline (speedup 1.0000x reference)
"""MoE top-2 dispatch -> per-expert Linear -> gated combine, on 8 TRN2 cores.

Strategy: data-parallel over the 16384-token batch (2048 tokens/core).
Host side does the *dispatch bookkeeping only* (zero FLOPs): per core,
(token, expert) pairs are sorted by expert into 128-padded segments and the
routed activations are laid out as a d-blocked, transposed tensor so the
device needs no transpose.  The device runs per-expert matmuls (top-2 sparse
compute), applies gate scaling on PSUM eviction, stores pair-ordered rows to
a DRAM scratch, then combines with a static pass: per output token-tile one
indirect gather of the token's two pair rows + vector add.

Self-contained: shapes hardcoded for B=16384, E=8, D=1024, O=1024, K=2.
"""

import os
import sys
import types

sys.path.insert(0, "/opt/trn_rl_repo")

import ml_dtypes
import numpy as np

import concourse.bass as bass
import concourse.mybir as mybir
from concourse import bass_utils
from concourse.tile import TileContext

B, E, D, O = 16384, 8, 1024, 1024
N_CORES = 8
BT = B // N_CORES  # tokens per core
P = 128
KO = D // P  # contraction chunks
OT = 512  # output tile (one PSUM bank of fp32)
NOT = O // OT
NTT = BT // P  # output token tiles per core

_DT_MAP = {
    "float16": (mybir.dt.float16, np.float16),
    "bfloat16": (mybir.dt.bfloat16, ml_dtypes.bfloat16),
    "float32r": (mybir.dt.float32r, np.float32),
    "float32": (mybir.dt.float32, np.float32),
}

MAX_WAITS = int(os.environ.get("MOE_MAX_WAITS", "1"))


def _patch_tile_drain():
    """Public-walrus workaround: walrus codegen rejects instructions carrying
    more than a couple of sync-wait commands.  Tile's add_semaphores can put
    several waits on one instruction (and the kernel-tail drain carries one
    per live processor).  Hoist excess waits onto single-wait nop carriers
    emitted just before the instruction on the same engine."""
    from concourse.tile import TileContext as TC
    from concourse.vector_clock import ScopedClock

    if getattr(TC, "_moe_drain_patched", False):
        return

    orig_add = TC._add_instruction

    def _add_instruction(self, inst):
        si = getattr(inst, "sync_info", None)
        waits = list(si.on_wait or []) if si is not None else []
        if len(waits) > MAX_WAITS:
            hoist = waits[: len(waits) - MAX_WAITS]
            keep = waits[len(waits) - MAX_WAITS :]
            for w in hoist:
                nop = mybir.InstNoOp(
                    name=self.nc.get_next_instruction_name(),
                    engine=inst.engine,
                    bass_nofuse=True,
                    sync_info=mybir.SyncInfo(on_wait=[w], on_update=[]),
                )
                orig_add(self, nop)
            inst.sync_info = mybir.SyncInfo(
                on_wait=keep, on_update=list(si.on_update or [])
            )
        orig_add(self, inst)

    def _drain_and_barrier(self, tick_clock, wait_clock):
        carrier = self.nc.sync.nop(nofuse=True)
        wait_clock.add_sem_waits(
            carrier.ins, ScopedClock({None: tick_clock.global_clock})
        )
        si = carrier.ins.sync_info
        waits = list(si.on_wait or []) if si is not None else []
        if len(waits) > 1:
            carrier.ins.sync_info = mybir.SyncInfo(
                on_wait=waits[:1], on_update=list(si.on_update or [])
            )
            for w in waits[1:]:
                extra = self.nc.sync.nop(nofuse=True)
                extra.ins.sync_info = mybir.SyncInfo(on_wait=[w], on_update=[])
        self.nc.sync.drain()
        self.nc.all_engine_barrier()
        assert self.sems is not None
        popped = self.nc._tile_sem_poison_stack.pop()
        assert popped is self._sem_poison
        self.nc.clear_and_free_semaphores(list(self.sems.allocated().values()))
        self.nc.all_engine_barrier()

    TC._add_instruction = _add_instruction
    TC._drain_and_barrier = _drain_and_barrier
    TC._moe_drain_patched = True


def _assign_tokens(gates):
    """Balanced token->core assignment: round-robin per expert-pair type so
    every (core, expert) segment is ~n_e/8, minimizing SPMD tile padding.
    Returns core_tokens[c] = sorted global token ids (len == BT each)."""
    exp = np.argsort(-gates, axis=1)[:, :2]  # two routed experts per token
    e1 = np.minimum(exp[:, 0], exp[:, 1])
    e2 = np.maximum(exp[:, 0], exp[:, 1])
    type_id = e1 * E + e2
    order = np.argsort(type_id, kind="stable")  # tokens grouped by type
    cores = np.empty(B, np.int64)
    cores[order] = np.arange(B) % N_CORES  # round-robin within each type
    # fix up counts to exactly BT per core (moves are rare and tiny)
    counts = np.bincount(cores, minlength=N_CORES)
    over = [c for c in range(N_CORES) if counts[c] > BT]
    under = [c for c in range(N_CORES) if counts[c] < BT]
    for c in over:
        surplus = counts[c] - BT
        victims = np.nonzero(cores == c)[0][:surplus]
        for v in victims:
            tgt = under[0]
            cores[v] = tgt
            counts[tgt] += 1
            counts[c] -= 1
            if counts[tgt] == BT:
                under.pop(0)
    assert (np.bincount(cores, minlength=N_CORES) == BT).all()
    cores = _swap_repair(cores, e1, e2)
    return [np.sort(np.nonzero(cores == c)[0]) for c in range(N_CORES)]


def _tile_total(cores, e1, e2):
    counts = np.zeros((N_CORES, E), np.int64)
    np.add.at(counts, (cores, e1), 1)
    np.add.at(counts, (cores, e2), 1)
    return int(np.ceil(np.sort(counts, 1)[:, ::-1] / P).max(0).sum()), counts


def _swap_repair(cores, e1, e2):
    """Concentrate each globally-oversized expert's surplus onto dedicated
    overflow cores via randomized token swaps, so most (core, expert)
    segments fit in 4 tiles (<=512) and only a few need 5 (<=640)."""
    base_T, counts = _tile_total(cores, e1, e2)
    n_e = counts.sum(0)
    surplus = n_e - N_CORES * 512
    need = [int(np.ceil(s / P)) for s in np.maximum(surplus, 0)]
    if sum(need) > N_CORES:
        return cores
    cap = np.full((N_CORES, E), 512, np.int64)
    free = list(range(N_CORES))
    for e in np.argsort(-surplus):
        for _ in range(need[e]):
            cap[free.pop(0), e] = 512 + P
    cur = cores.copy()
    rng = np.random.default_rng(0)
    by_core = [list(np.nonzero(cur == c)[0]) for c in range(N_CORES)]
    over = counts - cap

    def viol():
        return int(np.maximum(over, 0).sum())

    v = viol()
    for _ in range(60000):
        if v == 0:
            break
        cs, es = np.nonzero(over > 0)
        c, e = cs[0], es[0]
        cand = [t for t in rng.choice(by_core[c], size=min(64, BT), replace=False)
                if e1[t] == e or e2[t] == e]
        if not cand:
            break
        t = cand[0]
        d = int(rng.integers(N_CORES))
        if d == c:
            continue
        u = int(by_core[d][int(rng.integers(len(by_core[d])))])
        delta = np.zeros((N_CORES, E), np.int64)
        for tok, src, dst in ((t, c, d), (u, d, c)):
            for ee in (e1[tok], e2[tok]):
                delta[src, ee] -= 1
                delta[dst, ee] += 1
        new_over = over + delta
        if int(np.maximum(new_over, 0).sum()) < v:
            over = new_over
            v = int(np.maximum(over, 0).sum())
            by_core[c].remove(t)
            by_core[d].append(t)
            by_core[d].remove(u)
            by_core[c].append(u)
            cur[t], cur[u] = d, c
    new_T, _ = _tile_total(cur, e1, e2)
    return cur if new_T < base_T else cores


def _route(gates, core_tokens):
    """Per-core dispatch plan.  plans[c] = (perm, idxs, gs) with experts
    permuted largest-segment-first; k_pattern[s] = tile count of segment s
    (max over cores, so one SPMD program serves every core — per-core expert
    identity is handled by permuting W/b host-side)."""
    plans = []
    counts = np.zeros((N_CORES, E), np.int64)
    for c in range(N_CORES):
        gs = gates[core_tokens[c]]  # [BT, E]
        idxs = [np.nonzero(gs[:, e] > 0)[0].astype(np.int32) for e in range(E)]
        perm = np.argsort([-len(ix) for ix in idxs], kind="stable")
        plans.append((perm, idxs, gs))
        counts[c] = [len(idxs[e]) for e in perm]
    k_pattern = [int(np.ceil(counts[:, s].max() / P)) for s in range(E)]
    return plans, k_pattern


def _build_core_inputs(x, W, b, plan, k_pattern, np_dt, y_np_dt):
    perm, idxs, gs = plan
    T = sum(k_pattern)
    toks = np.zeros((T * P,), np.int64)  # gathered token (local) per pair slot
    gvals = np.zeros((T * P,), np.float32)
    real = np.zeros((T * P,), bool)
    t0 = 0
    for s in range(E):
        e = perm[s]
        ix = idxs[e]
        n = len(ix)
        toks[t0 : t0 + n] = ix
        gvals[t0 : t0 + n] = gs[ix, e]
        real[t0 : t0 + n] = True
        t0 += k_pattern[s] * P
    # combine indices: for each token its two pair rows (pair row = flat slot)
    pos = np.full((BT, 2), -1, np.int64)
    fill = np.zeros((BT,), np.int64)
    rr = np.nonzero(real)[0]
    for r in rr:
        tok = toks[r]
        pos[tok, fill[tok]] = r
        fill[tok] += 1
    assert (fill == 2).all(), "every token must have exactly 2 routed experts"
    comb = pos.reshape(NTT, P, 2).transpose(1, 0, 2).reshape(P, NTT * 2)
    # d-blocked transposed gather: xg[t, ki, ko, p] = x[tok(t,p), ko*128+ki]
    xg = x[toks].astype(np_dt).reshape(T, P, KO, P).transpose(0, 3, 2, 1).copy()
    # W blocked per (permuted) expert: w[e, ki, ko, o] = W[perm[e], ko*128+ki, o]
    wb = W[perm].astype(np_dt).reshape(E, KO, P, O).transpose(0, 2, 1, 3).copy()
    g_arr = gvals.reshape(T, P).T.copy()  # [P, T]
    bb = b[perm].astype(np_dt).reshape(1, E, O).copy()
    return {
        "xg": xg,
        "w": wb,
        "g": g_arr,
        "comb": comb.astype(np.int32),
        "bvec": bb,
    }


def _build_program_a(k_pattern, dt, ydt, bias_flag):
    """Compute NEFF: per-expert matmuls over gathered pairs, gate scale,
    store pair-ordered rows y[pair] = gate * (x @ W_e + b_e)."""
    T = sum(k_pattern)
    nc = bass.Bass(target_bir_lowering=False, trn_type="TRN2")
    xg_d = nc.dram_tensor("xg", [T, P, KO, P], dt, kind="ExternalInput")
    w_d = nc.dram_tensor("w", [E, P, KO, O], dt, kind="ExternalInput")
    g_d = nc.dram_tensor("g", [P, T], mybir.dt.float32, kind="ExternalInput")
    b_d = nc.dram_tensor("bvec", [1, E, O], dt, kind="ExternalInput")
    y_d = nc.dram_tensor("y", [T * P, O], ydt, kind="ExternalOutput")

    with TileContext(nc) as tc:
        with (
            tc.tile_pool(name="const", bufs=1) as cpool,
            tc.tile_pool(name="wp", bufs=3) as wpool,
            tc.tile_pool(name="xp", bufs=8) as xpool,
            tc.tile_pool(name="yt", bufs=6) as ypool,
            tc.tile_pool(name="ps", bufs=8, space="PSUM") as pspool,
        ):
            g_sb = cpool.tile([P, T], mybir.dt.float32)
            nc.sync.dma_start(out=g_sb[:], in_=g_d[:, :])
            if bias_flag:
                b_sb = cpool.tile([1, E, O], dt)
                nc.sync.dma_start(out=b_sb[:], in_=b_d[:, :, :])
                ones_sb = cpool.tile([1, P], dt)
                nc.vector.memset(ones_sb[:], 1.0)

            t = 0
            for s in range(E):
                ks = k_pattern[s]
                w_half = []
                for ot in range(NOT):
                    wt = wpool.tile([P, KO, OT], dt, tag=f"w{ot}")
                    nc.sync.dma_start(
                        out=wt[:], in_=w_d[s, :, :, ot * OT : (ot + 1) * OT]
                    )
                    w_half.append(wt)
                for _ in range(ks):
                    x_sb = xpool.tile([P, KO, P], dt, tag="x")
                    nc.sync.dma_start(out=x_sb[:], in_=xg_d[t, :, :, :])
                    y_sb = ypool.tile([P, O], ydt, tag="y")
                    for ot in range(NOT):
                        ps = pspool.tile([P, OT], mybir.dt.float32, tag="ps")
                        for ko in range(KO):
                            nc.tensor.matmul(
                                out=ps[:],
                                lhsT=x_sb[:, ko, :],
                                rhs=w_half[ot][:, ko, :],
                                start=(ko == 0),
                                stop=(ko == KO - 1 and not bias_flag),
                            )
                        if bias_flag:
                            nc.tensor.matmul(
                                out=ps[:],
                                lhsT=ones_sb[:1, :],
                                rhs=b_sb[:1, s, ot * OT : (ot + 1) * OT],
                                start=False,
                                stop=True,
                            )
                        nc.vector.tensor_scalar_mul(
                            out=y_sb[:, ot * OT : (ot + 1) * OT],
                            in0=ps[:],
                            scalar1=g_sb[:, t : t + 1],
                        )
                    nc.sync.dma_start(
                        out=y_d[t * P : (t + 1) * P, :], in_=y_sb[:]
                    )
                    t += 1
    return nc


def _build_program_b(T, ydt):
    """Combine NEFF: out[tok] = y[pairA(tok)] + y[pairB(tok)] via indirect
    gathers (y is a pristine input here — gather-from-written-tensor and
    indirect scatter are both broken under this runtime, hence two NEFFs)."""
    nc = bass.Bass(target_bir_lowering=False, trn_type="TRN2")
    y_d = nc.dram_tensor("y", [T * P, O], ydt, kind="ExternalInput")
    comb_d = nc.dram_tensor("comb", [P, NTT * 2], mybir.dt.int32,
                            kind="ExternalInput")
    out_d = nc.dram_tensor("out", [BT, O], mybir.dt.float32,
                           kind="ExternalOutput")
    with TileContext(nc) as tc:
        with (
            tc.tile_pool(name="const", bufs=1) as cpool,
            tc.tile_pool(name="ix", bufs=32) as ipool,
            tc.tile_pool(name="cb", bufs=14) as gpool,
        ):
            comb_sb = cpool.tile([P, NTT * 2], mybir.dt.int32)
            nc.sync.dma_start(out=comb_sb[:], in_=comb_d[:, :])
            for g in range(NTT):
                parts = []
                for sl in range(2):
                    # dedicated offset-0 index tile (indirect DMA drops
                    # the index AP's in-tile offset on hardware)
                    it = ipool.tile([P, 1], mybir.dt.int32, tag="it")
                    nc.vector.tensor_copy(
                        out=it[:], in_=comb_sb[:, 2 * g + sl : 2 * g + sl + 1]
                    )
                    gt = gpool.tile([P, O], ydt, tag=f"g{sl}")
                    nc.gpsimd.indirect_dma_start(
                        out=gt[:],
                        out_offset=None,
                        in_=y_d[:, :],
                        in_offset=bass.IndirectOffsetOnAxis(ap=it[:, :1], axis=0),
                    )
                    parts.append(gt)
                o_sb = gpool.tile([P, O], mybir.dt.float32, tag="osb")
                nc.vector.tensor_add(
                    out=o_sb[:], in0=parts[0][:], in1=parts[1][:]
                )
                nc.sync.dma_start(
                    out=out_d[g * P : (g + 1) * P, :], in_=o_sb[:]
                )
    return nc


def kernel(x, gates, W, b):
    _patch_tile_drain()
    dt_name = os.environ.get("MOE_DT", "float16")
    ydt_name = os.environ.get("MOE_YDT", "float16")
    dt, np_dt = _DT_MAP[dt_name]
    ydt, y_np_dt = _DT_MAP[ydt_name]
    bias_flag = bool(np.any(b != 0))

    gates = np.asarray(gates)
    x = np.ascontiguousarray(x)
    W = np.asarray(W)
    b = np.asarray(b)
    core_tokens = _assign_tokens(gates)
    plans, k_pattern = _route(gates, core_tokens)
    in_maps = []
    for c in range(N_CORES):
        xs = x[core_tokens[c]]
        in_maps.append(
            _build_core_inputs(xs, W, b, plans[c], k_pattern, np_dt, y_np_dt)
        )

    T = sum(k_pattern)
    nc_a = _build_program_a(k_pattern, dt, ydt, bias_flag)
    nc_b = _build_program_b(T, ydt)

    trace = os.environ.get("MOE_TRACE", "0") == "1"
    kwargs = {}
    if trace:
        _install_ntff_shim()
        kwargs = dict(trace=True, trace_cores=list(range(N_CORES)))

    in_maps_a = [
        {k: m[k] for k in ("xg", "w", "g", "bvec")} for m in in_maps
    ]
    res_a = bass_utils.run_bass_kernel_spmd(
        nc_a, in_maps_a, core_ids=list(range(N_CORES)), **kwargs
    )
    in_maps_b = [
        {"y": res_a.results[c]["y"], "comb": in_maps[c]["comb"]}
        for c in range(N_CORES)
    ]
    res_b = bass_utils.run_bass_kernel_spmd(
        nc_b, in_maps_b, core_ids=list(range(N_CORES)), **kwargs
    )
    if trace and res_a.exec_time_ns is not None and res_b.exec_time_ns is not None:
        total = res_a.exec_time_ns + res_b.exec_time_ns
        print(f"HW exec time: {total} ns "
              f"(compute {res_a.exec_time_ns} + combine {res_b.exec_time_ns}; "
              f"means {res_a.mean_exec_time_ns:.0f} + "
              f"{res_b.mean_exec_time_ns:.0f})")
    out = np.empty((B, O), np.float32)
    for c in range(N_CORES):
        out[core_tokens[c]] = res_b.results[c]["out"]
    return out


def _install_ntff_shim():
    """Best-effort: register the missing antenv.axon_hooks NTFF profile hook
    so trace=True yields exec_time_ns.  Only used when MOE_TRACE=1."""
    try:
        import antenv
        from trn_agent_boot.trn_boot import _ntff_profile_via_ctypes

        if "antenv.axon_hooks" in sys.modules:
            return
        hooks = types.ModuleType("antenv.axon_hooks")
        hook = _ntff_profile_via_ctypes("/opt/axon/libaxon_pjrt.so")
        hooks.get_axon_ntff_profile_hook = lambda: hook
        hooks.set_axon_ntff_profile_hook = lambda h: None
        sys.modules["antenv.axon_hooks"] = hooks
        antenv.axon_hooks = hooks
        bass_utils.upload_artifacts = lambda tmpdir: tmpdir
    except Exception as e:  # pragma: no cover
        print(f"ntff shim unavailable: {e}", file=sys.stderr)



# revision 10
# speedup vs baseline: 1.3887x; 1.3887x over previous
"""MoE top-2 dispatch -> per-expert Linear -> gated combine, on 8 TRN2 cores.

Single fused NEFF, transposed formulation.  Tokens are grouped by their
expert-pair "type" {e1,e2}; type {i,j} is split into one chunk on core i and
one on core j (star dispatch: every chunk on core c contains expert c), so a
core's groups all share slot-0 = its center expert.  Per group the device
computes out^T[o, tok] = g_a*(W_a^T x) + g_b*(W_b^T x) with tokens on the
matmul FREE dim (no 128-token tile quantization) and the top-2 combine done
for free by PSUM accumulation across the two experts.  Gates are folded into
x by a partition-broadcast (ones-matmul) + elementwise multiply on DVE.

Host side does dispatch bookkeeping only (gather/transpose/permute, zero
FLOPs).  Self-contained: shapes hardcoded for B=16384, E=8, D=1024, O=1024.
"""

import os
import sys
import types

sys.path.insert(0, "/opt/trn_rl_repo")

import ml_dtypes
import numpy as np

import concourse.bass as bass
import concourse.mybir as mybir
from concourse import bass_utils
from concourse.tile import TileContext

B, E, D, O = 16384, 8, 1024, 1024
N_CORES = 8
P = 128
KB = D // P   # contraction blocks (8)
NOB = O // P  # output row blocks (8)

_DT_MAP = {
    "float16": (mybir.dt.float16, np.float16),
    "bfloat16": (mybir.dt.bfloat16, ml_dtypes.bfloat16),
    "float32r": (mybir.dt.float32r, np.float32),
    "float32": (mybir.dt.float32, np.float32),
}

MAX_WAITS = int(os.environ.get("MOE_MAX_WAITS", "1"))


def _patch_tile_drain():
    """Public-walrus workaround: walrus codegen rejects instructions carrying
    more than a couple of sync-wait commands.  Tile's add_semaphores can put
    several waits on one instruction (and the kernel-tail drain carries one
    per live processor).  Hoist excess waits onto single-wait nop carriers
    emitted just before the instruction on the same engine."""
    from concourse.tile import TileContext as TC
    from concourse.vector_clock import ScopedClock

    if getattr(TC, "_moe_drain_patched", False):
        return

    orig_add = TC._add_instruction

    def _add_instruction(self, inst):
        si = getattr(inst, "sync_info", None)
        waits = list(si.on_wait or []) if si is not None else []
        if len(waits) > MAX_WAITS:
            hoist = waits[: len(waits) - MAX_WAITS]
            keep = waits[len(waits) - MAX_WAITS :]
            for w in hoist:
                nop = mybir.InstNoOp(
                    name=self.nc.get_next_instruction_name(),
                    engine=inst.engine,
                    bass_nofuse=True,
                    sync_info=mybir.SyncInfo(on_wait=[w], on_update=[]),
                )
                orig_add(self, nop)
            inst.sync_info = mybir.SyncInfo(
                on_wait=keep, on_update=list(si.on_update or [])
            )
        orig_add(self, inst)

    def _drain_and_barrier(self, tick_clock, wait_clock):
        carrier = self.nc.sync.nop(nofuse=True)
        wait_clock.add_sem_waits(
            carrier.ins, ScopedClock({None: tick_clock.global_clock})
        )
        si = carrier.ins.sync_info
        waits = list(si.on_wait or []) if si is not None else []
        if len(waits) > 1:
            carrier.ins.sync_info = mybir.SyncInfo(
                on_wait=waits[:1], on_update=list(si.on_update or [])
            )
            for w in waits[1:]:
                extra = self.nc.sync.nop(nofuse=True)
                extra.ins.sync_info = mybir.SyncInfo(on_wait=[w], on_update=[])
        self.nc.sync.drain()
        self.nc.all_engine_barrier()
        assert self.sems is not None
        popped = self.nc._tile_sem_poison_stack.pop()
        assert popped is self._sem_poison
        self.nc.clear_and_free_semaphores(list(self.sems.allocated().values()))
        self.nc.all_engine_barrier()

    TC._add_instruction = _add_instruction
    TC._drain_and_barrier = _drain_and_barrier
    TC._moe_drain_patched = True


_MASK = ~np.eye(E, dtype=bool)


def _profile(a):
    """Per-core chunk sizes sorted descending: [E, E-1]."""
    return -np.sort(-a[_MASK].reshape(E, E - 1), axis=1)


def _plan_splits(nmat):
    """Split each type {i,j} into chunks a[i,j] (on core i) and a[j,i]
    (on core j), minimizing CT = sum_k max_c (k-th largest chunk of core c),
    i.e. the canonical padded column count of the SPMD program.
    Simulated annealing over the 28 split points."""
    a0 = np.zeros((E, E), np.int64)
    for i in range(E):
        for j in range(i + 1, E):
            n = int(nmat[i, j])
            a0[i, j] = n // 2
            a0[j, i] = n - n // 2

    def ct_of(a):
        return int(_profile(a).max(0).sum())

    pairs = [(i, j) for i in range(E) for j in range(i + 1, E)]
    deltas = [1, -1, 2, -2, 4, -4, 8, -8, 16, -16, 32, -32, 64, -64]
    best_a, best_ct = a0.copy(), ct_of(a0)
    iters = int(os.environ.get("MOE_PLAN_ITERS", "150000"))
    for seed in range(2):
        rng = np.random.default_rng(seed)
        a = a0.copy()
        cur = float(ct_of(a))
        T0, T1 = 60.0, 0.05
        for t in range(iters):
            T = T0 * (T1 / T0) ** (t / iters)
            i, j = pairs[int(rng.integers(len(pairs)))]
            d = deltas[int(rng.integers(len(deltas)))]
            n = int(nmat[i, j])
            na = int(a[i, j]) + d
            if na < 0 or na > n:
                continue
            old = int(a[i, j])
            a[i, j] = na
            a[j, i] = n - na
            sc = float(ct_of(a))
            if sc <= cur or rng.random() < np.exp(-(sc - cur) / max(T, 1e-9)):
                cur = sc
                if sc < best_ct:
                    best_ct, best_a = int(sc), a.copy()
            else:
                a[i, j] = old
                a[j, i] = n - old
    return best_a, best_ct


def _route(gates):
    """Global dispatch plan.  Returns (plans, positions) where positions is
    the canonical group list [(slot, F)] (slot = partner W slot, 1-based;
    consecutive repeats share W) and plans[c] = (perm, tok_cols, g2, real):
      perm     : slot -> expert permutation (slot 0 = center = c)
      tok_cols : [CT] global token id per column (pads = 0)
      g2       : [2, CT] gate for slot-a (center) / slot-b (partner)
      real     : [CT] bool, True where the column is a real token
    """
    g = np.asarray(gates)
    order = np.argsort(-g, axis=1)[:, :2]
    e_lo = np.minimum(order[:, 0], order[:, 1])
    e_hi = np.maximum(order[:, 0], order[:, 1])
    nmat = np.zeros((E, E), np.int64)
    np.add.at(nmat, (e_lo, e_hi), 1)
    nmat = nmat + nmat.T

    a, _ = _plan_splits(nmat)

    # token lists per type; first a[i,j] tokens of {i,j} -> core i, rest -> j
    chunk_toks = {}
    for i in range(E):
        for j in range(i + 1, E):
            toks = np.nonzero((e_lo == i) & (e_hi == j))[0]
            ai = int(a[i, j])
            chunk_toks[(i, j)] = toks[:ai]
            chunk_toks[(j, i)] = toks[ai:]

    # canonical rank sizes: need[k] = max over cores of k-th largest chunk;
    # ranks > 512 split into equal sub-positions (PSUM bank = 512 fp32 cols)
    need = _profile(a).max(0)
    positions = []  # (rank k, slot k+1, F)
    for k in range(E - 1):
        n = int(need[k])
        if n <= 0:
            continue
        m = -(-n // 512)
        base, rem = divmod(n, m)
        for s in range(m):
            positions.append((k, k + 1, base + (1 if s < rem else 0)))
    CT = sum(f for _k, _s, f in positions)

    plans = []
    for c in range(E):
        partners = [p for p in range(E) if p != c]
        partners.sort(key=lambda p: -len(chunk_toks[(c, p)]))
        perm = [c] + partners
        tok_cols = np.zeros(CT, np.int64)
        g2 = np.zeros((2, CT), np.float32)
        real = np.zeros(CT, bool)
        cursor = [0] * (E - 1)
        off = 0
        for k, _slot, f in positions:
            p = partners[k]
            toks = chunk_toks[(c, p)][cursor[k] : cursor[k] + f]
            cursor[k] += len(toks)
            n = len(toks)
            tok_cols[off : off + n] = toks
            g2[0, off : off + n] = g[toks, c]
            g2[1, off : off + n] = g[toks, p]
            real[off : off + n] = True
            off += f
        assert all(
            cursor[k] == len(chunk_toks[(c, partners[k])]) for k in range(E - 1)
        ), "unplaced tokens"
        plans.append((perm, tok_cols, g2, real))
    return plans, [(s, f) for _k, s, f in positions]


def _build_core_inputs(x, W, b, plan, positions, np_dt, bias_flag):
    perm, tok_cols, g2, _real = plan
    CT = len(tok_cols)
    xt = (
        x[tok_cols]
        .astype(np_dt)
        .reshape(CT, KB, P)
        .transpose(2, 1, 0)
        .copy()
    )  # [128(ki), KB, CT]
    w = (
        W[perm]
        .astype(np_dt)
        .reshape(E, KB, P, O)
        .transpose(0, 2, 1, 3)
        .copy()
    )  # [slot, 128(ki), KB, O]
    m = {"xt": xt, "w": w, "g2": g2.astype(np_dt)}
    if bias_flag:
        G = len(positions)
        b2 = np.zeros((2, G, O), np.float32)
        b2[0, :, :] = b[perm[0]]
        for k, (slot, _f) in enumerate(positions):
            b2[1, k, :] = b[perm[slot]]
        m["b2"] = b2.astype(np_dt)
    return m


def _build_program(positions, dt, bias_flag):
    """One fused NEFF: per group k (columns c0:c0+F[k]) accumulate in PSUM
    out^T[o_block] = W_slot0^T (x*g_a) + W_slotk^T (x*g_b) (+ bias via a
    rank-2 matmul with the gate rows), evict through the scalar engine."""
    G = len(positions)
    slots = [s for s, _f in positions]
    F = [f for _s, f in positions]
    CT = sum(F)
    OBP = 256  # W dma chunk along O (2 o-blocks)
    nc = bass.Bass(target_bir_lowering=False, trn_type="TRN2")
    xt_d = nc.dram_tensor("xt", [P, KB, CT], dt, kind="ExternalInput")
    w_d = nc.dram_tensor("w", [E, P, KB, O], dt, kind="ExternalInput")
    g_d = nc.dram_tensor("g2", [2, CT], dt, kind="ExternalInput")
    if bias_flag:
        b_d = nc.dram_tensor("b2", [2, G, O], dt, kind="ExternalInput")
    out_d = nc.dram_tensor("out", [NOB, P, CT], dt, kind="ExternalOutput")

    offs = np.concatenate([[0], np.cumsum(F)])

    with TileContext(nc) as tc:
        with (
            tc.tile_pool(name="const", bufs=1) as cpool,
            tc.tile_pool(name="wp", bufs=3) as wpool,
            tc.tile_pool(name="xtp", bufs=3) as xtpool,
            tc.tile_pool(name="xg", bufs=32) as xgpool,
            tc.tile_pool(name="gs", bufs=4) as gspool,
            tc.tile_pool(name="ot", bufs=6) as opool,
            tc.tile_pool(name="psg", bufs=3, space="PSUM") as gppool,
            tc.tile_pool(name="ps", bufs=4, space="PSUM") as pspool,
        ):
            ones = cpool.tile([1, P], dt)
            nc.vector.memset(ones[:], 1.0)
            # separate tiles per gate row: matmul rhs base partition must be 0
            ga_sb = cpool.tile([1, CT], dt)
            nc.sync.dma_start(out=ga_sb[:], in_=g_d[0:1, :])
            gb_sb = cpool.tile([1, CT], dt)
            nc.sync.dma_start(out=gb_sb[:], in_=g_d[1:2, :])
            grows = (ga_sb, gb_sb)
            if bias_flag:
                g_sb = cpool.tile([2, CT], dt)
                nc.sync.dma_start(out=g_sb[:], in_=g_d[:, :])
                b_sb = cpool.tile([2, G, O], dt)
                nc.sync.dma_start(out=b_sb[:], in_=b_d[:, :, :])
            # center expert weights, resident; split along O for startup
            w0 = []
            for obp in range(O // OBP):
                w0t = cpool.tile([P, KB, OBP], dt, name=f"w0_{obp}")
                nc.sync.dma_start(
                    out=w0t[:], in_=w_d[0, :, :, obp * OBP : (obp + 1) * OBP]
                )
                w0.append(w0t)

            xt_t = [None] * G
            w_t = [None] * G
            xg_t = [None] * G

            def prepare(k):
                c0, f = int(offs[k]), F[k]
                xt = xtpool.tile([P, KB, f], dt, tag="xt")
                nc.sync.dma_start(out=xt[:], in_=xt_d[:, :, c0 : c0 + f])
                xt_t[k] = xt
                if k > 0 and slots[k] == slots[k - 1]:
                    w_t[k] = w_t[k - 1]  # sub-position: same partner W
                else:
                    wts = []
                    for obp in range(O // OBP):
                        wt = wpool.tile([P, KB, OBP], dt, tag=f"w{obp}")
                        nc.sync.dma_start(
                            out=wt[:],
                            in_=w_d[slots[k], :, :, obp * OBP : (obp + 1) * OBP],
                        )
                        wts.append(wt)
                    w_t[k] = wts
                xgs = []
                for s in range(2):
                    gp = gppool.tile([P, f], mybir.dt.float32, tag="G")
                    nc.tensor.matmul(
                        out=gp[:],
                        lhsT=ones[:1, :],
                        rhs=grows[s][:1, c0 : c0 + f],
                        start=True,
                        stop=True,
                    )
                    gs = gspool.tile([P, f], dt, tag="Gs")
                    nc.vector.tensor_copy(out=gs[:], in_=gp[:])
                    row = []
                    for kb in range(KB):
                        xg = xgpool.tile([P, f], dt, tag="xg")
                        nc.vector.tensor_mul(
                            out=xg[:], in0=xt[:, kb, :], in1=gs[:]
                        )
                        row.append(xg)
                    xgs.append(row)
                xg_t[k] = xgs

            def compute(k):
                c0, f = int(offs[k]), F[k]
                xgs = xg_t[k]
                for ob in range(NOB):
                    obp, half = divmod(ob * P, OBP)
                    ps = pspool.tile([P, f], mybir.dt.float32, tag="ps")
                    for kb in range(KB):
                        nc.tensor.matmul(
                            out=ps[:],
                            lhsT=w0[obp][:, kb, half : half + P],
                            rhs=xgs[0][kb][:],
                            start=(kb == 0),
                            stop=False,
                        )
                    for kb in range(KB):
                        nc.tensor.matmul(
                            out=ps[:],
                            lhsT=w_t[k][obp][:, kb, half : half + P],
                            rhs=xgs[1][kb][:],
                            start=False,
                            stop=(kb == KB - 1 and not bias_flag),
                        )
                    if bias_flag:
                        nc.tensor.matmul(
                            out=ps[:],
                            lhsT=b_sb[0:2, k, ob * P : (ob + 1) * P],
                            rhs=g_sb[0:2, c0 : c0 + f],
                            start=False,
                            stop=True,
                        )
                    o_t = opool.tile([P, f], dt, tag="o")
                    nc.scalar.copy(out=o_t[:], in_=ps[:])
                    nc.sync.dma_start(
                        out=out_d[ob, :, c0 : c0 + f], in_=o_t[:]
                    )
                # release references so pools can recycle
                xg_t[k] = None
                w_t[k] = None
                xt_t[k] = None

            prepare(0)
            if G > 1:
                prepare(1)
            for k in range(G):
                compute(k)
                if k + 2 < G:
                    prepare(k + 2)
    return nc


def kernel(x, gates, W, b):
    _patch_tile_drain()
    dt_name = os.environ.get("MOE_DT", "float16")
    dt, np_dt = _DT_MAP[dt_name]
    bias_flag = bool(np.any(b != 0))

    gates = np.asarray(gates)
    x = np.ascontiguousarray(x)
    W = np.asarray(W)
    b = np.asarray(b)

    plans, positions = _route(gates)
    in_maps = [
        _build_core_inputs(x, W, b, plans[c], positions, np_dt, bias_flag)
        for c in range(N_CORES)
    ]
    nc = _build_program(positions, dt, bias_flag)

    trace = os.environ.get("MOE_TRACE", "0") == "1"
    kwargs = {}
    if trace:
        _install_ntff_shim()
        kwargs = dict(trace=True, trace_cores=list(range(N_CORES)))

    res = bass_utils.run_bass_kernel_spmd(
        nc, in_maps, core_ids=list(range(N_CORES)), **kwargs
    )
    if trace and res.exec_time_ns is not None:
        print(
            f"HW exec time: {res.exec_time_ns} ns "
            f"(mean {res.mean_exec_time_ns:.0f})"
        )
    out = np.empty((B, O), np.float32)
    for c in range(N_CORES):
        perm, tok_cols, _g2, real = plans[c]
        arr = res.results[c]["out"].reshape(O, len(tok_cols))
        out[tok_cols[real]] = arr[:, real].T.astype(np.float32)
    return out


def _install_ntff_shim():
    """Best-effort: register the missing antenv.axon_hooks NTFF profile hook
    so trace=True yields exec_time_ns.  Only used when MOE_TRACE=1."""
    try:
        import antenv
        from trn_agent_boot.trn_boot import _ntff_profile_via_ctypes

        if "antenv.axon_hooks" in sys.modules:
            return
        hooks = types.ModuleType("antenv.axon_hooks")
        hook = _ntff_profile_via_ctypes("/opt/axon/libaxon_pjrt.so")
        hooks.get_axon_ntff_profile_hook = lambda: hook
        hooks.set_axon_ntff_profile_hook = lambda h: None
        sys.modules["antenv.axon_hooks"] = hooks
        antenv.axon_hooks = hooks
        bass_utils.upload_artifacts = lambda tmpdir: tmpdir
    except Exception as e:  # pragma: no cover
        print(f"ntff shim unavailable: {e}", file=sys.stderr)


# revision 13
# speedup vs baseline: 1.4127x; 1.0173x over previous
"""MoE top-2 dispatch -> per-expert Linear -> gated combine, on 8 TRN2 cores.

Single fused NEFF, transposed formulation.  Tokens are grouped by their
expert-pair "type" {e1,e2}; type {i,j} is split into one chunk on core i and
one on core j (star dispatch: every chunk on core c contains expert c), so a
core's groups all share slot-0 = its center expert.  Per group the device
computes out^T[o, tok] = g_a*(W_a^T x) + g_b*(W_b^T x) with tokens on the
matmul FREE dim (no 128-token tile quantization) and the top-2 combine done
for free by PSUM accumulation across the two experts.  Gates are folded into
x by a partition-broadcast (ones-matmul) + elementwise multiply on DVE.

Host side does dispatch bookkeeping only (gather/transpose/permute, zero
FLOPs).  Self-contained: shapes hardcoded for B=16384, E=8, D=1024, O=1024.
"""

import os
import sys
import types

sys.path.insert(0, "/opt/trn_rl_repo")

import ml_dtypes
import numpy as np

import concourse.bass as bass
import concourse.mybir as mybir
from concourse import bass_utils
from concourse.tile import TileContext

B, E, D, O = 16384, 8, 1024, 1024
N_CORES = 8
P = 128
KB = D // P   # contraction blocks (8)
NOB = O // P  # output row blocks (8)

_DT_MAP = {
    "float16": (mybir.dt.float16, np.float16),
    "bfloat16": (mybir.dt.bfloat16, ml_dtypes.bfloat16),
    "float32r": (mybir.dt.float32r, np.float32),
    "float32": (mybir.dt.float32, np.float32),
}

MAX_WAITS = int(os.environ.get("MOE_MAX_WAITS", "1"))


def _patch_tile_drain():
    """Public-walrus workaround: walrus codegen rejects instructions carrying
    more than a couple of sync-wait commands.  Tile's add_semaphores can put
    several waits on one instruction (and the kernel-tail drain carries one
    per live processor).  Hoist excess waits onto single-wait nop carriers
    emitted just before the instruction on the same engine."""
    from concourse.tile import TileContext as TC
    from concourse.vector_clock import ScopedClock

    if getattr(TC, "_moe_drain_patched", False):
        return

    orig_add = TC._add_instruction

    def _add_instruction(self, inst):
        si = getattr(inst, "sync_info", None)
        waits = list(si.on_wait or []) if si is not None else []
        if len(waits) > MAX_WAITS:
            hoist = waits[: len(waits) - MAX_WAITS]
            keep = waits[len(waits) - MAX_WAITS :]
            for w in hoist:
                nop = mybir.InstNoOp(
                    name=self.nc.get_next_instruction_name(),
                    engine=inst.engine,
                    bass_nofuse=True,
                    sync_info=mybir.SyncInfo(on_wait=[w], on_update=[]),
                )
                orig_add(self, nop)
            inst.sync_info = mybir.SyncInfo(
                on_wait=keep, on_update=list(si.on_update or [])
            )
        orig_add(self, inst)

    def _drain_and_barrier(self, tick_clock, wait_clock):
        carrier = self.nc.sync.nop(nofuse=True)
        wait_clock.add_sem_waits(
            carrier.ins, ScopedClock({None: tick_clock.global_clock})
        )
        si = carrier.ins.sync_info
        waits = list(si.on_wait or []) if si is not None else []
        if len(waits) > 1:
            carrier.ins.sync_info = mybir.SyncInfo(
                on_wait=waits[:1], on_update=list(si.on_update or [])
            )
            for w in waits[1:]:
                extra = self.nc.sync.nop(nofuse=True)
                extra.ins.sync_info = mybir.SyncInfo(on_wait=[w], on_update=[])
        self.nc.sync.drain()
        self.nc.all_engine_barrier()
        assert self.sems is not None
        popped = self.nc._tile_sem_poison_stack.pop()
        assert popped is self._sem_poison
        self.nc.clear_and_free_semaphores(list(self.sems.allocated().values()))
        self.nc.all_engine_barrier()

    TC._add_instruction = _add_instruction
    TC._drain_and_barrier = _drain_and_barrier
    TC._moe_drain_patched = True


_MASK = ~np.eye(E, dtype=bool)


def _profile(a):
    """Per-core chunk sizes sorted descending: [E, E-1]."""
    return -np.sort(-a[_MASK].reshape(E, E - 1), axis=1)


def _plan_splits(nmat):
    """Split each type {i,j} into chunks a[i,j] (on core i) and a[j,i]
    (on core j), minimizing CT = sum_k max_c (k-th largest chunk of core c),
    i.e. the canonical padded column count of the SPMD program.
    Simulated annealing over the 28 split points."""
    a0 = np.zeros((E, E), np.int64)
    for i in range(E):
        for j in range(i + 1, E):
            n = int(nmat[i, j])
            a0[i, j] = n // 2
            a0[j, i] = n - n // 2

    def ct_of(a):
        return int(_profile(a).max(0).sum())

    pairs = [(i, j) for i in range(E) for j in range(i + 1, E)]
    deltas = [1, -1, 2, -2, 4, -4, 8, -8, 16, -16, 32, -32, 64, -64]
    best_a, best_ct = a0.copy(), ct_of(a0)
    iters = int(os.environ.get("MOE_PLAN_ITERS", "150000"))
    for seed in range(2):
        rng = np.random.default_rng(seed)
        a = a0.copy()
        cur = float(ct_of(a))
        T0, T1 = 60.0, 0.05
        for t in range(iters):
            T = T0 * (T1 / T0) ** (t / iters)
            i, j = pairs[int(rng.integers(len(pairs)))]
            d = deltas[int(rng.integers(len(deltas)))]
            n = int(nmat[i, j])
            na = int(a[i, j]) + d
            if na < 0 or na > n:
                continue
            old = int(a[i, j])
            a[i, j] = na
            a[j, i] = n - na
            sc = float(ct_of(a))
            if sc <= cur or rng.random() < np.exp(-(sc - cur) / max(T, 1e-9)):
                cur = sc
                if sc < best_ct:
                    best_ct, best_a = int(sc), a.copy()
            else:
                a[i, j] = old
                a[j, i] = n - old
    return best_a, best_ct


def _route(gates):
    """Global dispatch plan.  Returns (plans, positions) where positions is
    the canonical group list [(slot, F)] (slot = partner W slot, 1-based;
    consecutive repeats share W) and plans[c] = (perm, tok_cols, g2, real):
      perm     : slot -> expert permutation (slot 0 = center = c)
      tok_cols : [CT] global token id per column (pads = 0)
      g2       : [2, CT] gate for slot-a (center) / slot-b (partner)
      real     : [CT] bool, True where the column is a real token
    """
    g = np.asarray(gates)
    order = np.argsort(-g, axis=1)[:, :2]
    e_lo = np.minimum(order[:, 0], order[:, 1])
    e_hi = np.maximum(order[:, 0], order[:, 1])
    nmat = np.zeros((E, E), np.int64)
    np.add.at(nmat, (e_lo, e_hi), 1)
    nmat = nmat + nmat.T

    a, _ = _plan_splits(nmat)

    # token lists per type; first a[i,j] tokens of {i,j} -> core i, rest -> j
    chunk_toks = {}
    for i in range(E):
        for j in range(i + 1, E):
            toks = np.nonzero((e_lo == i) & (e_hi == j))[0]
            ai = int(a[i, j])
            chunk_toks[(i, j)] = toks[:ai]
            chunk_toks[(j, i)] = toks[ai:]

    # canonical rank sizes: need[k] = max over cores of k-th largest chunk;
    # ranks > 512 split into equal sub-positions (PSUM bank = 512 fp32 cols)
    need = _profile(a).max(0)
    positions = []  # (rank k, slot k+1, F)
    for k in range(E - 1):
        n = int(need[k])
        if n <= 0:
            continue
        m = -(-n // 512)
        base, rem = divmod(n, m)
        for s in range(m):
            positions.append((k, k + 1, base + (1 if s < rem else 0)))
    CT = sum(f for _k, _s, f in positions)

    plans = []
    for c in range(E):
        partners = [p for p in range(E) if p != c]
        partners.sort(key=lambda p: -len(chunk_toks[(c, p)]))
        perm = [c] + partners
        tok_cols = np.zeros(CT, np.int64)
        g2 = np.zeros((2, CT), np.float32)
        real = np.zeros(CT, bool)
        cursor = [0] * (E - 1)
        off = 0
        for k, _slot, f in positions:
            p = partners[k]
            toks = chunk_toks[(c, p)][cursor[k] : cursor[k] + f]
            cursor[k] += len(toks)
            n = len(toks)
            tok_cols[off : off + n] = toks
            g2[0, off : off + n] = g[toks, c]
            g2[1, off : off + n] = g[toks, p]
            real[off : off + n] = True
            off += f
        assert all(
            cursor[k] == len(chunk_toks[(c, partners[k])]) for k in range(E - 1)
        ), "unplaced tokens"
        plans.append((perm, tok_cols, g2, real))
    return plans, [(s, f) for _k, s, f in positions]


def _build_core_inputs(x, W, b, plan, positions, np_dt, bias_flag):
    perm, tok_cols, g2, _real = plan
    CT = len(tok_cols)
    xt = (
        x[tok_cols]
        .astype(np_dt)
        .reshape(CT, KB, P)
        .transpose(2, 1, 0)
        .copy()
    )  # [128(ki), KB, CT]
    w = (
        W[perm]
        .astype(np_dt)
        .reshape(E, KB, P, O)
        .transpose(0, 2, 1, 3)
        .copy()
    )  # [slot, 128(ki), KB, O]
    m = {"xt": xt, "w": w, "g2": g2.astype(np_dt)}
    if bias_flag:
        G = len(positions)
        b2 = np.zeros((2, G, O), np.float32)
        b2[0, :, :] = b[perm[0]]
        for k, (slot, _f) in enumerate(positions):
            b2[1, k, :] = b[perm[slot]]
        m["b2"] = b2.astype(np_dt)
    return m


def _build_program(positions, dt, bias_flag):
    """One fused NEFF: per group k (columns c0:c0+F[k]) accumulate in PSUM
    out^T[o_block] = W_slot0^T (x*g_a) + W_slotk^T (x*g_b) (+ bias via a
    rank-2 matmul with the gate rows), evict through the scalar engine."""
    G = len(positions)
    slots = [s for s, _f in positions]
    F = [f for _s, f in positions]
    CT = sum(F)
    OBP = 256  # W dma chunk along O (2 o-blocks)
    nc = bass.Bass(target_bir_lowering=False, trn_type="TRN2")
    xt_d = nc.dram_tensor("xt", [P, KB, CT], dt, kind="ExternalInput")
    w_d = nc.dram_tensor("w", [E, P, KB, O], dt, kind="ExternalInput")
    g_d = nc.dram_tensor("g2", [2, CT], dt, kind="ExternalInput")
    if bias_flag:
        b_d = nc.dram_tensor("b2", [2, G, O], dt, kind="ExternalInput")
    out_d = nc.dram_tensor("out", [NOB, P, CT], dt, kind="ExternalOutput")

    offs = np.concatenate([[0], np.cumsum(F)])

    with TileContext(nc) as tc:
        with (
            tc.tile_pool(name="const", bufs=1) as cpool,
            tc.tile_pool(name="wp", bufs=3) as wpool,
            tc.tile_pool(name="xtp", bufs=3) as xtpool,
            tc.tile_pool(name="xg", bufs=32) as xgpool,
            tc.tile_pool(name="gs", bufs=4) as gspool,
            tc.tile_pool(name="ot", bufs=6) as opool,
            tc.tile_pool(name="psg", bufs=3, space="PSUM") as gppool,
            tc.tile_pool(name="ps", bufs=4, space="PSUM") as pspool,
        ):
            ones = cpool.tile([1, P], dt)
            nc.vector.memset(ones[:], 1.0)
            # separate tiles per gate row: matmul rhs base partition must be 0
            ga_sb = cpool.tile([1, CT], dt)
            nc.sync.dma_start(out=ga_sb[:], in_=g_d[0:1, :])
            gb_sb = cpool.tile([1, CT], dt)
            nc.sync.dma_start(out=gb_sb[:], in_=g_d[1:2, :])
            grows = (ga_sb, gb_sb)
            if bias_flag:
                g_sb = cpool.tile([2, CT], dt)
                nc.sync.dma_start(out=g_sb[:], in_=g_d[:, :])
                b_sb = cpool.tile([2, G, O], dt)
                nc.sync.dma_start(out=b_sb[:], in_=b_d[:, :, :])
            # center expert weights, resident; split along O so only the
            # first chunk gates the first matmul
            w0 = [None] * (O // OBP)

            def load_w0(obp):
                w0t = cpool.tile([P, KB, OBP], dt, name=f"w0_{obp}")
                nc.scalar.dma_start(
                    out=w0t[:], in_=w_d[0, :, :, obp * OBP : (obp + 1) * OBP]
                )
                w0[obp] = w0t

            xt_t = [None] * G
            w_t = [None] * G
            xg_t = [None] * G

            def prepare_w(k, obps):
                if k > 0 and slots[k] == slots[k - 1]:
                    w_t[k] = w_t[k - 1]  # sub-position: same partner W
                    return
                if w_t[k] is None:
                    w_t[k] = [None] * (O // OBP)
                for obp in obps:
                    wt = wpool.tile([P, KB, OBP], dt, tag=f"w{obp}")
                    nc.sync.dma_start(
                        out=wt[:],
                        in_=w_d[slots[k], :, :, obp * OBP : (obp + 1) * OBP],
                    )
                    w_t[k][obp] = wt

            def prepare_xg(k):
                c0, f = int(offs[k]), F[k]
                xt = xtpool.tile([P, KB, f], dt, tag="xt")
                nc.scalar.dma_start(out=xt[:], in_=xt_d[:, :, c0 : c0 + f])
                xt_t[k] = xt
                xgs = []
                for s in range(2):
                    gp = gppool.tile([P, f], mybir.dt.float32, tag="G")
                    nc.tensor.matmul(
                        out=gp[:],
                        lhsT=ones[:1, :],
                        rhs=grows[s][:1, c0 : c0 + f],
                        start=True,
                        stop=True,
                    )
                    gs = gspool.tile([P, f], dt, tag="Gs")
                    nc.vector.tensor_copy(out=gs[:], in_=gp[:])
                    row = []
                    for kb in range(KB):
                        xg = xgpool.tile([P, f], dt, tag="xg")
                        nc.vector.tensor_mul(
                            out=xg[:], in0=xt[:, kb, :], in1=gs[:]
                        )
                        row.append(xg)
                    xgs.append(row)
                xg_t[k] = xgs

            def compute(k):
                c0, f = int(offs[k]), F[k]
                xgs = xg_t[k]
                for ob in range(NOB):
                    obp, half = divmod(ob * P, OBP)
                    ps = pspool.tile([P, f], mybir.dt.float32, tag="ps")
                    for kb in range(KB):
                        nc.tensor.matmul(
                            out=ps[:],
                            lhsT=w0[obp][:, kb, half : half + P],
                            rhs=xgs[0][kb][:],
                            start=(kb == 0),
                            stop=False,
                        )
                    for kb in range(KB):
                        nc.tensor.matmul(
                            out=ps[:],
                            lhsT=w_t[k][obp][:, kb, half : half + P],
                            rhs=xgs[1][kb][:],
                            start=False,
                            stop=(kb == KB - 1 and not bias_flag),
                        )
                    if bias_flag:
                        nc.tensor.matmul(
                            out=ps[:],
                            lhsT=b_sb[0:2, k, ob * P : (ob + 1) * P],
                            rhs=g_sb[0:2, c0 : c0 + f],
                            start=False,
                            stop=True,
                        )
                    o_t = opool.tile([P, f], dt, tag="o")
                    # alternate eviction engine: scalar / vector
                    if ob % 2 == 0:
                        nc.scalar.copy(out=o_t[:], in_=ps[:])
                    else:
                        nc.vector.tensor_copy(out=o_t[:], in_=ps[:])
                    nc.gpsimd.dma_start(
                        out=out_d[ob, :, c0 : c0 + f], in_=o_t[:]
                    )
                # release references so pools can recycle
                xg_t[k] = None
                w_t[k] = None
                xt_t[k] = None

            # startup: first xt + first W chunks gate the first matmuls;
            # stream the rest behind them
            rest = list(range(1, O // OBP))
            prepare_w(0, [0])
            load_w0(0)
            prepare_xg(0)
            for obp in rest:
                prepare_w(0, [obp])
                load_w0(obp)
            if G > 1:
                prepare_w(1, range(O // OBP))
                prepare_xg(1)
            for k in range(G):
                compute(k)
                if k + 2 < G:
                    prepare_w(k + 2, range(O // OBP))
                    prepare_xg(k + 2)
    return nc


def kernel(x, gates, W, b):
    _patch_tile_drain()
    dt_name = os.environ.get("MOE_DT", "float16")
    dt, np_dt = _DT_MAP[dt_name]
    bias_flag = bool(np.any(b != 0))

    gates = np.asarray(gates)
    x = np.ascontiguousarray(x)
    W = np.asarray(W)
    b = np.asarray(b)

    plans, positions = _route(gates)
    in_maps = [
        _build_core_inputs(x, W, b, plans[c], positions, np_dt, bias_flag)
        for c in range(N_CORES)
    ]
    nc = _build_program(positions, dt, bias_flag)

    trace = os.environ.get("MOE_TRACE", "0") == "1"
    kwargs = {}
    if trace:
        _install_ntff_shim()
        kwargs = dict(trace=True, trace_cores=list(range(N_CORES)))

    res = bass_utils.run_bass_kernel_spmd(
        nc, in_maps, core_ids=list(range(N_CORES)), **kwargs
    )
    if trace and res.exec_time_ns is not None:
        print(
            f"HW exec time: {res.exec_time_ns} ns "
            f"(mean {res.mean_exec_time_ns:.0f})"
        )
    out = np.empty((B, O), np.float32)
    for c in range(N_CORES):
        perm, tok_cols, _g2, real = plans[c]
        arr = res.results[c]["out"].reshape(O, len(tok_cols))
        out[tok_cols[real]] = arr[:, real].T.astype(np.float32)
    return out


def _install_ntff_shim():
    """Best-effort: register the missing antenv.axon_hooks NTFF profile hook
    so trace=True yields exec_time_ns.  Only used when MOE_TRACE=1."""
    try:
        import antenv
        from trn_agent_boot.trn_boot import _ntff_profile_via_ctypes

        if "antenv.axon_hooks" in sys.modules:
            return
        hooks = types.ModuleType("antenv.axon_hooks")
        hook = _ntff_profile_via_ctypes("/opt/axon/libaxon_pjrt.so")
        hooks.get_axon_ntff_profile_hook = lambda: hook
        hooks.set_axon_ntff_profile_hook = lambda h: None
        sys.modules["antenv.axon_hooks"] = hooks
        antenv.axon_hooks = hooks
        bass_utils.upload_artifacts = lambda tmpdir: tmpdir
    except Exception as e:  # pragma: no cover
        print(f"ntff shim unavailable: {e}", file=sys.stderr)


# revision 16
# speedup vs baseline: 1.4448x; 1.0227x over previous
"""MoE top-2 dispatch -> per-expert Linear -> gated combine, on 8 TRN2 cores.

Single fused NEFF, transposed formulation.  Tokens are grouped by their
expert-pair "type" {e1,e2}; type {i,j} is split into one chunk on core i and
one on core j (star dispatch: every chunk on core c contains expert c), so a
core's groups all share slot-0 = its center expert.  Per group the device
computes out^T[o, tok] = g_a*(W_a^T x) + g_b*(W_b^T x) with tokens on the
matmul FREE dim (no 128-token tile quantization) and the top-2 combine done
for free by PSUM accumulation across the two experts.  Gates are folded into
x by a partition-broadcast (ones-matmul) + elementwise multiply on DVE.

Host side does dispatch bookkeeping only (gather/transpose/permute, zero
FLOPs).  Self-contained: shapes hardcoded for B=16384, E=8, D=1024, O=1024.
"""

import os
import sys
import types

sys.path.insert(0, "/opt/trn_rl_repo")

import ml_dtypes
import numpy as np

import concourse.bass as bass
import concourse.mybir as mybir
from concourse import bass_utils
from concourse.tile import TileContext

B, E, D, O = 16384, 8, 1024, 1024
N_CORES = 8
P = 128
KB = D // P   # contraction blocks (8)
NOB = O // P  # output row blocks (8)

_DT_MAP = {
    "float16": (mybir.dt.float16, np.float16),
    "bfloat16": (mybir.dt.bfloat16, ml_dtypes.bfloat16),
    "float32r": (mybir.dt.float32r, np.float32),
    "float32": (mybir.dt.float32, np.float32),
}

MAX_WAITS = int(os.environ.get("MOE_MAX_WAITS", "1"))


def _patch_tile_drain():
    """Public-walrus workaround: walrus codegen rejects instructions carrying
    more than a couple of sync-wait commands.  Tile's add_semaphores can put
    several waits on one instruction (and the kernel-tail drain carries one
    per live processor).  Hoist excess waits onto single-wait nop carriers
    emitted just before the instruction on the same engine."""
    from concourse.tile import TileContext as TC
    from concourse.vector_clock import ScopedClock

    if getattr(TC, "_moe_drain_patched", False):
        return

    orig_add = TC._add_instruction

    def _add_instruction(self, inst):
        si = getattr(inst, "sync_info", None)
        waits = list(si.on_wait or []) if si is not None else []
        if len(waits) > MAX_WAITS:
            hoist = waits[: len(waits) - MAX_WAITS]
            keep = waits[len(waits) - MAX_WAITS :]
            for w in hoist:
                nop = mybir.InstNoOp(
                    name=self.nc.get_next_instruction_name(),
                    engine=inst.engine,
                    bass_nofuse=True,
                    sync_info=mybir.SyncInfo(on_wait=[w], on_update=[]),
                )
                orig_add(self, nop)
            inst.sync_info = mybir.SyncInfo(
                on_wait=keep, on_update=list(si.on_update or [])
            )
        orig_add(self, inst)

    def _drain_and_barrier(self, tick_clock, wait_clock):
        carrier = self.nc.sync.nop(nofuse=True)
        wait_clock.add_sem_waits(
            carrier.ins, ScopedClock({None: tick_clock.global_clock})
        )
        si = carrier.ins.sync_info
        waits = list(si.on_wait or []) if si is not None else []
        if len(waits) > 1:
            carrier.ins.sync_info = mybir.SyncInfo(
                on_wait=waits[:1], on_update=list(si.on_update or [])
            )
            for w in waits[1:]:
                extra = self.nc.sync.nop(nofuse=True)
                extra.ins.sync_info = mybir.SyncInfo(on_wait=[w], on_update=[])
        self.nc.sync.drain()
        self.nc.all_engine_barrier()
        assert self.sems is not None
        popped = self.nc._tile_sem_poison_stack.pop()
        assert popped is self._sem_poison
        self.nc.clear_and_free_semaphores(list(self.sems.allocated().values()))
        self.nc.all_engine_barrier()

    TC._add_instruction = _add_instruction
    TC._drain_and_barrier = _drain_and_barrier
    TC._moe_drain_patched = True


_MASK = ~np.eye(E, dtype=bool)


def _profile(a):
    """Per-core chunk sizes sorted descending: [E, E-1]."""
    return -np.sort(-a[_MASK].reshape(E, E - 1), axis=1)


def _plan_splits(nmat):
    """Split each type {i,j} into chunks a[i,j] (on core i) and a[j,i]
    (on core j), minimizing CT = sum_k max_c (k-th largest chunk of core c),
    i.e. the canonical padded column count of the SPMD program.
    Simulated annealing over the 28 split points."""
    a0 = np.zeros((E, E), np.int64)
    for i in range(E):
        for j in range(i + 1, E):
            n = int(nmat[i, j])
            a0[i, j] = n // 2
            a0[j, i] = n - n // 2

    def ct_of(a):
        return int(_profile(a).max(0).sum())

    pairs = [(i, j) for i in range(E) for j in range(i + 1, E)]
    deltas = [1, -1, 2, -2, 4, -4, 8, -8, 16, -16, 32, -32, 64, -64]
    best_a, best_ct = a0.copy(), ct_of(a0)
    iters = int(os.environ.get("MOE_PLAN_ITERS", "150000"))
    for seed in range(2):
        rng = np.random.default_rng(seed)
        a = a0.copy()
        cur = float(ct_of(a))
        T0, T1 = 60.0, 0.05
        for t in range(iters):
            T = T0 * (T1 / T0) ** (t / iters)
            i, j = pairs[int(rng.integers(len(pairs)))]
            d = deltas[int(rng.integers(len(deltas)))]
            n = int(nmat[i, j])
            na = int(a[i, j]) + d
            if na < 0 or na > n:
                continue
            old = int(a[i, j])
            a[i, j] = na
            a[j, i] = n - na
            sc = float(ct_of(a))
            if sc <= cur or rng.random() < np.exp(-(sc - cur) / max(T, 1e-9)):
                cur = sc
                if sc < best_ct:
                    best_ct, best_a = int(sc), a.copy()
            else:
                a[i, j] = old
                a[j, i] = n - old
    return best_a, best_ct


def _route(gates):
    """Global dispatch plan.  Returns (plans, positions) where positions is
    the canonical group list [(slot, F)] (slot = partner W slot, 1-based;
    consecutive repeats share W) and plans[c] = (perm, tok_cols, g2, real):
      perm     : slot -> expert permutation (slot 0 = center = c)
      tok_cols : [CT] global token id per column (pads = 0)
      g2       : [2, CT] gate for slot-a (center) / slot-b (partner)
      real     : [CT] bool, True where the column is a real token
    """
    g = np.asarray(gates)
    order = np.argsort(-g, axis=1)[:, :2]
    e_lo = np.minimum(order[:, 0], order[:, 1])
    e_hi = np.maximum(order[:, 0], order[:, 1])
    nmat = np.zeros((E, E), np.int64)
    np.add.at(nmat, (e_lo, e_hi), 1)
    nmat = nmat + nmat.T

    a, _ = _plan_splits(nmat)

    # token lists per type; first a[i,j] tokens of {i,j} -> core i, rest -> j
    chunk_toks = {}
    for i in range(E):
        for j in range(i + 1, E):
            toks = np.nonzero((e_lo == i) & (e_hi == j))[0]
            ai = int(a[i, j])
            chunk_toks[(i, j)] = toks[:ai]
            chunk_toks[(j, i)] = toks[ai:]

    # canonical rank sizes: need[k] = max over cores of k-th largest chunk;
    # ranks > 512 split into equal sub-positions (PSUM bank = 512 fp32 cols)
    need = _profile(a).max(0)
    positions = []  # (rank k, slot k+1, F)
    for k in range(E - 1):
        n = int(need[k])
        if n <= 0:
            continue
        m = -(-n // 512)
        base, rem = divmod(n, m)
        for s in range(m):
            positions.append((k, k + 1, base + (1 if s < rem else 0)))
    CT = sum(f for _k, _s, f in positions)

    plans = []
    for c in range(E):
        partners = [p for p in range(E) if p != c]
        partners.sort(key=lambda p: -len(chunk_toks[(c, p)]))
        perm = [c] + partners
        tok_cols = np.zeros(CT, np.int64)
        g2 = np.zeros((2, CT), np.float32)
        real = np.zeros(CT, bool)
        cursor = [0] * (E - 1)
        off = 0
        for k, _slot, f in positions:
            p = partners[k]
            toks = chunk_toks[(c, p)][cursor[k] : cursor[k] + f]
            cursor[k] += len(toks)
            n = len(toks)
            tok_cols[off : off + n] = toks
            g2[0, off : off + n] = g[toks, c]
            g2[1, off : off + n] = g[toks, p]
            real[off : off + n] = True
            off += f
        assert all(
            cursor[k] == len(chunk_toks[(c, partners[k])]) for k in range(E - 1)
        ), "unplaced tokens"
        plans.append((perm, tok_cols, g2, real))
    return plans, [(s, f) for _k, s, f in positions]


def _build_core_inputs(x, W, b, plan, positions, np_dt, bias_flag):
    perm, tok_cols, g2, _real = plan
    CT = len(tok_cols)
    xt = (
        x[tok_cols]
        .astype(np_dt)
        .reshape(CT, KB, P)
        .transpose(2, 1, 0)
        .copy()
    )  # [128(ki), KB, CT]
    w = (
        W[perm]
        .astype(np_dt)
        .reshape(E, KB, P, O)
        .transpose(0, 2, 1, 3)
        .copy()
    )  # [slot, 128(ki), KB, O]
    m = {"xt": xt, "w": w, "g2": g2.astype(np_dt)}
    if bias_flag:
        G = len(positions)
        b2 = np.zeros((2, G, O), np.float32)
        b2[0, :, :] = b[perm[0]]
        for k, (slot, _f) in enumerate(positions):
            b2[1, k, :] = b[perm[slot]]
        m["b2"] = b2.astype(np_dt)
    return m


def _build_program(positions, dt, bias_flag):
    """One fused NEFF: per group k (columns c0:c0+F[k]) accumulate in PSUM
    out^T[o_block] = W_slot0^T (x*g_a) + W_slotk^T (x*g_b) (+ bias via a
    rank-2 matmul with the gate rows), evict through the scalar engine."""
    G = len(positions)
    slots = [s for s, _f in positions]
    F = [f for _s, f in positions]
    CT = sum(F)
    OBP = 256  # W dma chunk along O (2 o-blocks)
    nc = bass.Bass(target_bir_lowering=False, trn_type="TRN2")
    xt_d = nc.dram_tensor("xt", [P, KB, CT], dt, kind="ExternalInput")
    w_d = nc.dram_tensor("w", [E, P, KB, O], dt, kind="ExternalInput")
    g_d = nc.dram_tensor("g2", [2, CT], dt, kind="ExternalInput")
    if bias_flag:
        b_d = nc.dram_tensor("b2", [2, G, O], dt, kind="ExternalInput")
    out_d = nc.dram_tensor("out", [P, NOB, CT], dt, kind="ExternalOutput")

    offs = np.concatenate([[0], np.cumsum(F)])

    with TileContext(nc) as tc:
        with (
            tc.tile_pool(name="const", bufs=1) as cpool,
            tc.tile_pool(name="wp", bufs=3) as wpool,
            tc.tile_pool(name="xtp", bufs=3) as xtpool,
            tc.tile_pool(name="xg", bufs=32) as xgpool,
            tc.tile_pool(name="gs", bufs=4) as gspool,
            tc.tile_pool(name="ot", bufs=6) as opool,
            tc.tile_pool(name="psg", bufs=3, space="PSUM") as gppool,
            tc.tile_pool(name="ps", bufs=4, space="PSUM") as pspool,
        ):
            ones = cpool.tile([1, P], dt)
            nc.vector.memset(ones[:], 1.0)
            # separate tiles per gate row: matmul rhs base partition must be 0
            ga_sb = cpool.tile([1, CT], dt)
            nc.sync.dma_start(out=ga_sb[:], in_=g_d[0:1, :])
            gb_sb = cpool.tile([1, CT], dt)
            nc.sync.dma_start(out=gb_sb[:], in_=g_d[1:2, :])
            grows = (ga_sb, gb_sb)
            if bias_flag:
                g_sb = cpool.tile([2, CT], dt)
                nc.sync.dma_start(out=g_sb[:], in_=g_d[:, :])
                b_sb = cpool.tile([2, G, O], dt)
                nc.sync.dma_start(out=b_sb[:], in_=b_d[:, :, :])
            # center expert weights, resident; split along O so only the
            # first chunk gates the first matmul
            w0 = [None] * (O // OBP)

            def load_w0(obp):
                w0t = cpool.tile([P, KB, OBP], dt, name=f"w0_{obp}")
                nc.scalar.dma_start(
                    out=w0t[:], in_=w_d[0, :, :, obp * OBP : (obp + 1) * OBP]
                )
                w0[obp] = w0t

            xt_t = [None] * G
            w_t = [None] * G
            xg_t = [None] * G

            def prepare_w(k, obps):
                if k > 0 and slots[k] == slots[k - 1]:
                    w_t[k] = w_t[k - 1]  # sub-position: same partner W
                    return
                if w_t[k] is None:
                    w_t[k] = [None] * (O // OBP)
                for obp in obps:
                    wt = wpool.tile([P, KB, OBP], dt, tag=f"w{obp}")
                    nc.sync.dma_start(
                        out=wt[:],
                        in_=w_d[slots[k], :, :, obp * OBP : (obp + 1) * OBP],
                    )
                    w_t[k][obp] = wt

            def prepare_xg(k):
                c0, f = int(offs[k]), F[k]
                xt = xtpool.tile([P, KB, f], dt, tag="xt")
                nc.scalar.dma_start(out=xt[:], in_=xt_d[:, :, c0 : c0 + f])
                xt_t[k] = xt
                xgs = []
                for s in range(2):
                    gp = gppool.tile([P, f], mybir.dt.float32, tag="G")
                    nc.tensor.matmul(
                        out=gp[:],
                        lhsT=ones[:1, :],
                        rhs=grows[s][:1, c0 : c0 + f],
                        start=True,
                        stop=True,
                    )
                    gs = gspool.tile([P, f], dt, tag="Gs")
                    nc.vector.tensor_copy(out=gs[:], in_=gp[:])
                    row = []
                    for kb in range(KB):
                        xg = xgpool.tile([P, f], dt, tag="xg")
                        nc.vector.tensor_mul(
                            out=xg[:], in0=xt[:, kb, :], in1=gs[:]
                        )
                        row.append(xg)
                    xgs.append(row)
                xg_t[k] = xgs

            def compute(k):
                c0, f = int(offs[k]), F[k]
                xgs = xg_t[k]
                o_t = opool.tile([P, NOB, f], dt, tag="o")
                for ob in range(NOB):
                    obp, half = divmod(ob * P, OBP)
                    ps = pspool.tile([P, f], mybir.dt.float32, tag="ps")
                    for kb in range(KB):
                        nc.tensor.matmul(
                            out=ps[:],
                            lhsT=w0[obp][:, kb, half : half + P],
                            rhs=xgs[0][kb][:],
                            start=(kb == 0),
                            stop=False,
                        )
                    for kb in range(KB):
                        nc.tensor.matmul(
                            out=ps[:],
                            lhsT=w_t[k][obp][:, kb, half : half + P],
                            rhs=xgs[1][kb][:],
                            start=False,
                            stop=(kb == KB - 1 and not bias_flag),
                        )
                    if bias_flag:
                        nc.tensor.matmul(
                            out=ps[:],
                            lhsT=b_sb[0:2, k, ob * P : (ob + 1) * P],
                            rhs=g_sb[0:2, c0 : c0 + f],
                            start=False,
                            stop=True,
                        )
                    # alternate eviction engine: scalar / vector
                    if ob % 2 == 0:
                        nc.scalar.copy(out=o_t[:, ob, :], in_=ps[:])
                    else:
                        nc.vector.tensor_copy(out=o_t[:, ob, :], in_=ps[:])
                nc.sync.dma_start(out=out_d[:, :, c0 : c0 + f], in_=o_t[:])
                # release references so pools can recycle
                xg_t[k] = None
                w_t[k] = None
                xt_t[k] = None

            # startup: first xt + first W chunks gate the first matmuls;
            # stream the rest behind them
            rest = list(range(1, O // OBP))
            prepare_w(0, [0])
            load_w0(0)
            prepare_xg(0)
            for obp in rest:
                prepare_w(0, [obp])
                load_w0(obp)
            if G > 1:
                prepare_w(1, range(O // OBP))
                prepare_xg(1)
            for k in range(G):
                compute(k)
                if k + 2 < G:
                    prepare_w(k + 2, range(O // OBP))
                    prepare_xg(k + 2)
    return nc


def kernel(x, gates, W, b):
    _patch_tile_drain()
    dt_name = os.environ.get("MOE_DT", "float16")
    dt, np_dt = _DT_MAP[dt_name]
    bias_flag = bool(np.any(b != 0))

    gates = np.asarray(gates)
    x = np.ascontiguousarray(x)
    W = np.asarray(W)
    b = np.asarray(b)

    plans, positions = _route(gates)
    in_maps = [
        _build_core_inputs(x, W, b, plans[c], positions, np_dt, bias_flag)
        for c in range(N_CORES)
    ]
    nc = _build_program(positions, dt, bias_flag)

    trace = os.environ.get("MOE_TRACE", "0") == "1"
    kwargs = {}
    if trace:
        _install_ntff_shim()
        kwargs = dict(trace=True, trace_cores=list(range(N_CORES)))

    res = bass_utils.run_bass_kernel_spmd(
        nc, in_maps, core_ids=list(range(N_CORES)), **kwargs
    )
    if trace and res.exec_time_ns is not None:
        print(
            f"HW exec time: {res.exec_time_ns} ns "
            f"(mean {res.mean_exec_time_ns:.0f})"
        )
    out = np.empty((B, O), np.float32)
    for c in range(N_CORES):
        perm, tok_cols, _g2, real = plans[c]
        CT = len(tok_cols)
        arr = (
            res.results[c]["out"]
            .reshape(P, NOB, CT)
            .transpose(1, 0, 2)
            .reshape(O, CT)
        )
        out[tok_cols[real]] = arr[:, real].T.astype(np.float32)
    return out


def _install_ntff_shim():
    """Best-effort: register the missing antenv.axon_hooks NTFF profile hook
    so trace=True yields exec_time_ns.  Only used when MOE_TRACE=1."""
    try:
        import antenv
        from trn_agent_boot.trn_boot import _ntff_profile_via_ctypes

        if "antenv.axon_hooks" in sys.modules:
            return
        hooks = types.ModuleType("antenv.axon_hooks")
        hook = _ntff_profile_via_ctypes("/opt/axon/libaxon_pjrt.so")
        hooks.get_axon_ntff_profile_hook = lambda: hook
        hooks.set_axon_ntff_profile_hook = lambda h: None
        sys.modules["antenv.axon_hooks"] = hooks
        antenv.axon_hooks = hooks
        bass_utils.upload_artifacts = lambda tmpdir: tmpdir
    except Exception as e:  # pragma: no cover
        print(f"ntff shim unavailable: {e}", file=sys.stderr)
